# revision 11
# baseline (speedup 1.0000x reference)
"""Trainium2 Bass kernel for nn_DistHead (block-diagonal molecule attention).

out = softmax_blockdiag(Q K^T / sqrt(H)) * exp(-invr0 * cdist(Z, Z)) @ V
with Q/K/V = X @ W{q,k,v}^T, block-diagonal over 128 molecules of 64 atoms.

Sharding: 16 whole molecules (1024 rows) per core across 8 cores --
perfectly parallel, zero cross-core communication.

Key structure (v1 rewrite):
- Scores computed TRANSPOSED (lhsT=K^T, rhs=Q^T) so exp(ST) is directly
  the PV stationary -- no PE transposes, no psum->sbuf weight copies.
- Block-diag mask: one K=1 matmul adding sig_i*sig_j (+-256) to each
  score tile, plus bias=-256 folded into the exp -- off-block scores
  underflow to exactly 0.
- Row sums of exp(S) as N=1 matmuls against a ones column (PE), not DVE
  reduces.
- dist chain: sqrt(invr0^2*d2 + eps) via ACT free affine (clamp fused),
  then exp(-x). Sqrt table preloaded by a dummy activation during the
  DMA wait.
- Q/K projections stacked into one [128,512] psum (stationary [Wq|Wk]),
  halving the psum->sbuf cast traffic.
- PE warmed up by dummy matmuls during the input-DMA wait so real
  matmuls run at 2.4 GHz.
- 3 input DMA issues total (zz, xt, w); sig/ones constants via memset.

Self-contained: hardcodes shapes from the problem spec; only imports
concourse from /opt/trn_rl_repo.
"""

import sys

if "/opt/trn_rl_repo" not in sys.path:
    sys.path.insert(0, "/opt/trn_rl_repo")

import numpy as np

N, E, H = 8192, 256, 64          # atoms, embedding, head size
NSEG, SEG = 128, 64              # molecules, atoms per molecule
NCORES = 8
RPC = N // NCORES                # rows per core (1024 = 16 molecules)
NT = RPC // 128                  # 128-row tiles per core (2 molecules each)
HF = NT // 2                     # tiles per half
EC = E // 128                    # embedding chunks of 128
EPS = 5e-5                       # sqrt clamp bias (covers d2 roundoff)

_cache = {}


def _build_nc():
    import concourse.bacc as bacc
    import concourse.tile as tile
    from concourse import mybir

    f32 = mybir.dt.float32
    f16 = mybir.dt.float16
    AF = mybir.ActivationFunctionType

    nc = bacc.Bacc(None, target_bir_lowering=False, debug=False)

    # zat = invr0*[z2,1,-2z], zbt = invr0*[1,z2,z] (dist matmul operands;
    # separate tensors: matmul lhsT/rhs must share base partition 0)
    zat_d = nc.dram_tensor("zat", [5, NT, 128], f32, kind="ExternalInput")
    zbt_d = nc.dram_tensor("zbt", [5, NT, 128], f32, kind="ExternalInput")
    xt_d = nc.dram_tensor("xt", [128, EC, RPC], f16, kind="ExternalInput")
    # w: [:, c, 0:64] = Wq^T*H^-0.5, [:, c, 64:128] = Wk^T, [:, c, 128:192] = Wv^T
    w_d = nc.dram_tensor("w", [128, EC, 192], f16, kind="ExternalInput")
    y_d = nc.dram_tensor("y", [RPC, H], f16, kind="ExternalOutput")

    with tile.TileContext(nc) as tc:
        with (
            tc.tile_pool(name="consts", bufs=1) as consts,
            tc.tile_pool(name="pd", bufs=1, space="PSUM") as pd,
            tc.tile_pool(name="pst", bufs=2, space="PSUM") as pst,
            tc.tile_pool(name="pmm", bufs=2, space="PSUM") as pmm,
            tc.tile_pool(name="po", bufs=2, space="PSUM") as po,
        ):
            # --- tiny on-device constants (no DMA) ---
            sig = consts.tile([1, 128], f16, tag="sig")
            nc.gpsimd.memset(sig[:, 0:SEG], 16.0)
            nc.gpsimd.memset(sig[:, SEG:128], -16.0)
            ones_col = consts.tile([128, 1], f16, tag="ones")
            nc.gpsimd.memset(ones_col, 1.0)
            wu_l = consts.tile([1, 128], f32, tag="wu_l")
            nc.gpsimd.memset(wu_l, 0.0)
            wu_r = consts.tile([1, 512], f32, tag="wu_r")
            nc.gpsimd.memset(wu_r, 0.0)
            sq_in = consts.tile([1, 1], f32, tag="sq_in")
            nc.gpsimd.memset(sq_in, 1.0)
            sq_out = consts.tile([1, 1], f32, tag="sq_out")
            eps_b = consts.tile([128, 1], f32, tag="eps_b")
            nc.gpsimd.memset(eps_b, EPS)
            nb256 = consts.tile([128, 1], f32, tag="nb256")
            nc.gpsimd.memset(nb256, -256.0)

            # --- input DMAs: zat/zbt first (dist chain gates on them) ---
            zat = consts.tile([5, NT, 128], f32, tag="zat")
            nc.sync.dma_start(out=zat, in_=zat_d[:, :, :])
            zbt = consts.tile([5, NT, 128], f32, tag="zbt")
            nc.scalar.dma_start(out=zbt, in_=zbt_d[:, :, :])
            xt = consts.tile([128, EC, RPC], f16, tag="xt")
            nc.sync.dma_start(out=xt, in_=xt_d[:, :, :])
            w_sb = consts.tile([128, EC, 192], f16, tag="w")
            nc.scalar.dma_start(out=w_sb, in_=w_d[:, :, :])

            # Preload the sqrt ACT table during the DMA wait.
            nc.scalar.activation(out=sq_out, in_=sq_in, func=AF.Sqrt)

            # --- PE warmup: keep the PE busy ~3.4us so the HAM clock
            # gate opens before the real matmuls arrive. ---
            with tc.high_priority():
                for i in range(4):
                    wu_ps = pmm.tile([128, 512], f32, tag="mi", name=f"wu{i}")
                    nc.tensor.matmul(wu_ps, lhsT=wu_l, rhs=wu_r, start=True, stop=True)

                # --- dist: d2 tiles via K=5 matmuls, then fused
                # sqrt(invr0^2*d2 + eps) and exp(-x). ---
                d_ps = pd.tile([128, NT, 128], f32, tag="d")
                for t in range(NT):
                    nc.tensor.matmul(
                        d_ps[:, t, :],
                        lhsT=zat[:, t, :],
                        rhs=zbt[:, t, :],
                        start=True,
                        stop=True,
                    )
                dist = consts.tile([128, NT, 128], f32, tag="dist")
                nc.scalar.activation(out=dist, in_=d_ps, func=AF.Sqrt, bias=eps_b)
                dexp = consts.tile([128, NT, 128], f16, tag="dexp")
                nc.scalar.activation(out=dexp, in_=dist, func=AF.Exp, scale=-1.0)

            # --- Q/K projections, stacked [Q^T; K^T] in one psum; split
            # into separate qt/kt tiles (matmul operands need base 0). ---
            qt = consts.tile([64, RPC], f16, tag="qt")
            kt = consts.tile([64, RPC], f16, tag="kt")
            for h in range(2):
                hs = slice(h * 512, (h + 1) * 512)
                p = pmm.tile([128, 512], f32, tag="mi", name=f"qk{h}")
                for c in range(EC):
                    nc.tensor.matmul(
                        p,
                        lhsT=w_sb[:, c, 0:128],
                        rhs=xt[:, c, hs],
                        start=(c == 0),
                        stop=(c == EC - 1),
                    )
                nc.vector.tensor_copy(out=qt[:, hs], in_=p[0:64, :])
                nc.vector.tensor_copy(out=kt[:, hs], in_=p[64:128, :])

            # --- V projection, grouped 4 tiles per psum bank ---
            v_sb = consts.tile([128, NT, H], f16, tag="v")
            for g in range(2):
                vp = pmm.tile([128, HF, H], f32, tag="mi", name=f"v{g}")
                for tl in range(HF):
                    t = g * HF + tl
                    for c in range(EC):
                        nc.tensor.matmul(
                            vp[:, tl, :],
                            lhsT=xt[:, c, t * 128 : (t + 1) * 128],
                            rhs=w_sb[:, c, 128:192],
                            start=(c == 0),
                            stop=(c == EC - 1),
                        )
                nc.vector.tensor_copy(out=v_sb[:, g * HF : (g + 1) * HF, :], in_=vp)

            # --- scores (transposed) + mask, exp, decay-mul, rowsums,
            # PV, normalize -- per half of 4 tiles. ---
            eT = consts.tile([128, NT, 128], f16, tag="eT")
            weiT = consts.tile([128, NT, 128], f16, tag="weiT")
            rinv = consts.tile([128, NT], f32, tag="rinv")
            y_sb = consts.tile([128, NT, H], f16, tag="y")
            y_r = y_d.rearrange("(t p) h -> p t h", p=128)

            st_ps = [None] * 2
            for h in range(2):
                st_ps[h] = pst.tile([128, HF, 128], f32, tag="st", name=f"st{h}")
                for tl in range(HF):
                    t = h * HF + tl
                    rt = slice(t * 128, (t + 1) * 128)
                    nc.tensor.matmul(
                        st_ps[h][:, tl, :],
                        lhsT=kt[:, rt],
                        rhs=qt[:, rt],
                        start=True,
                        stop=False,
                    )
                    nc.tensor.matmul(
                        st_ps[h][:, tl, :],
                        lhsT=sig,
                        rhs=sig,
                        start=False,
                        stop=True,
                    )

            for h in range(2):
                hs = slice(h * HF, (h + 1) * HF)
                # exp(S-256): in-block mask contributes +256, off-block -256
                nc.scalar.activation(
                    out=eT[:, hs, :], in_=st_ps[h], func=AF.Exp, bias=nb256
                )
                nc.vector.tensor_mul(
                    out=weiT[:, hs, :], in0=eT[:, hs, :], in1=dexp[:, hs, :]
                )
                # rowsums: r_i = sum_j eT[j, i] via N=1 matmuls
                rs = pst.tile([128, HF], f32, tag="st", name=f"rs{h}")
                for tl in range(HF):
                    t = h * HF + tl
                    nc.tensor.matmul(
                        rs[:, tl : tl + 1],
                        lhsT=eT[:, t, :],
                        rhs=ones_col,
                        start=True,
                        stop=True,
                    )
                nc.vector.reciprocal(out=rinv[:, hs], in_=rs)
                o_ps = po.tile([128, HF, H], f32, tag="o", name=f"o{h}")
                for tl in range(HF):
                    t = h * HF + tl
                    nc.tensor.matmul(
                        o_ps[:, tl, :],
                        lhsT=weiT[:, t, :],
                        rhs=v_sb[:, t, :],
                        start=True,
                        stop=True,
                    )
                rb = rinv[:, hs].unsqueeze(2).broadcast_to([128, HF, H])
                nc.vector.tensor_mul(out=y_sb[:, hs, :], in0=o_ps, in1=rb)
                nc.sync.dma_start(out=y_r[:, hs, :], in_=y_sb[:, hs, :])

    nc.compile()
    return nc


def _get_nc():
    if "nc" not in _cache:
        _cache["nc"] = _build_nc()
    return _cache["nc"]


def _prepare_in_maps(X, Z, Wk, Wq, Wv, invr0):
    X = np.ascontiguousarray(X, dtype=np.float32)
    Z = np.ascontiguousarray(Z, dtype=np.float32)
    # [128, EC, N] fp16: partition p, chunk c -> X^T row c*128+p.
    xt_full = np.ascontiguousarray(
        X.T.reshape(EC, 128, N).transpose(1, 0, 2).astype(np.float16)
    )

    # invr0 folded into both dist operands: psum d2' = invr0^2 * d2, so
    # sqrt(d2' + eps) = invr0*dist and the decay is exp(-1.0 * x).
    inv = np.float32(np.asarray(invr0).reshape(-1)[0])
    z2 = np.sum(Z * Z, axis=-1)
    ones = np.ones(N, dtype=np.float32)
    zt = np.ascontiguousarray(Z.T)
    zat_full = (inv * np.concatenate([z2[None], ones[None], -2.0 * zt], axis=0)).astype(
        np.float32
    )
    zbt_full = (inv * np.concatenate([ones[None], z2[None], zt], axis=0)).astype(
        np.float32
    )

    scale = np.float32(H) ** np.float32(-0.5)
    # w: [128, EC, 192] = [Wq^T*scale | Wk^T | Wv^T] per chunk
    wq = (Wq.T * scale).astype(np.float32).reshape(EC, 128, H)
    wk = Wk.T.astype(np.float32).reshape(EC, 128, H)
    wv = Wv.T.astype(np.float32).reshape(EC, 128, H)
    w_full = np.ascontiguousarray(
        np.concatenate([wq, wk, wv], axis=2).astype(np.float16)
    )  # [EC, 128, 192] -> want [128, EC, 192]
    w_full = np.ascontiguousarray(w_full.transpose(1, 0, 2))

    in_maps = []
    for d in range(NCORES):
        s, e = d * RPC, (d + 1) * RPC
        in_maps.append(
            {
                "zat": np.ascontiguousarray(zat_full[:, s:e].reshape(5, NT, 128)),
                "zbt": np.ascontiguousarray(zbt_full[:, s:e].reshape(5, NT, 128)),
                "xt": np.ascontiguousarray(xt_full[:, :, s:e]),
                "w": w_full,
            }
        )
    return in_maps


def _run(in_maps, trace=False, **kwargs):
    from concourse.bass_utils import run_bass_kernel_spmd

    nc = _get_nc()
    return run_bass_kernel_spmd(nc, in_maps, list(range(NCORES)), trace=trace, **kwargs)


def _numpy_fallback(X, Z, Wk, Wq, Wv, invr0, ptr):
    """Reference-exact fallback for ptr layouts other than 128 x 64."""
    X = np.asarray(X, dtype=np.float32)
    Z = np.asarray(Z, dtype=np.float32)
    n = X.shape[0]
    K = X @ Wk.T
    Q = X @ Wq.T
    V = X @ Wv.T
    seg = np.searchsorted(np.asarray(ptr)[1:], np.arange(n), side="right")
    out = np.zeros((n, Wk.shape[0]), dtype=np.float32)
    inv = float(np.asarray(invr0).reshape(-1)[0])
    hs = Wk.shape[0] ** -0.5
    for s in np.unique(seg):
        idx = np.nonzero(seg == s)[0]
        q, k, v, z = Q[idx], K[idx], V[idx], Z[idx]
        wei = (q @ k.T) * hs
        wei = wei - wei.max(axis=-1, keepdims=True)
        wei = np.exp(wei)
        wei /= wei.sum(axis=-1, keepdims=True)
        d2 = np.maximum(
            (z * z).sum(-1)[:, None] + (z * z).sum(-1)[None, :] - 2.0 * (z @ z.T), 0.0
        )
        dist = np.sqrt(np.where(d2 > 0, d2, 1.0)) * (d2 > 0)
        wei = wei * np.exp(-inv * dist)
        out[idx] = wei @ v
    return out


def kernel(X, Z, Wk, Wq, Wv, invr0, ptr):
    ptr = np.asarray(ptr)
    if not (
        X.shape == (N, E)
        and Wk.shape == (H, E)
        and ptr.shape == (NSEG + 1,)
        and np.array_equal(ptr, np.arange(NSEG + 1, dtype=ptr.dtype) * SEG)
    ):
        return _numpy_fallback(X, Z, Wk, Wq, Wv, invr0, ptr)

    in_maps = _prepare_in_maps(X, Z, Wk, Wq, Wv, invr0)
    res = _run(in_maps, trace=False)
    out = np.empty((N, H), dtype=np.float32)
    for d in range(NCORES):
        out[d * RPC : (d + 1) * RPC] = res.results[d]["y"].astype(np.float32)
    return out


# revision 23
# speedup vs baseline: 1.2634x; 1.2634x over previous
"""Trainium2 Bass kernel for nn_DistHead (block-diagonal molecule attention).

out = softmax_blockdiag(Q K^T / sqrt(H)) * exp(-invr0 * cdist(Z, Z)) @ V
with Q/K/V = X @ W{q,k,v}^T, block-diagonal over 128 molecules of 64 atoms.

Sharding: 16 whole molecules (1024 rows) per core across 8 cores --
perfectly parallel, zero cross-core communication.

Key structure (v1 rewrite):
- Scores computed TRANSPOSED (lhsT=K^T, rhs=Q^T) so exp(ST) is directly
  the PV stationary -- no PE transposes, no psum->sbuf weight copies.
- Block-diag mask: one K=1 matmul adding sig_i*sig_j (+-256) to each
  score tile, plus bias=-256 folded into the exp -- off-block scores
  underflow to exactly 0.
- Row sums of exp(S) as N=1 matmuls against a ones column (PE), not DVE
  reduces.
- dist chain: sqrt(invr0^2*d2 + eps) via ACT free affine (clamp fused),
  then exp(-x). Sqrt table preloaded by a dummy activation during the
  DMA wait.
- Q/K projections stacked into one [128,512] psum (stationary [Wq|Wk]),
  halving the psum->sbuf cast traffic.
- PE warmed up by dummy matmuls during the input-DMA wait so real
  matmuls run at 2.4 GHz.
- 3 input DMA issues total (zz, xt, w); sig/ones constants via memset.

Self-contained: hardcodes shapes from the problem spec; only imports
concourse from /opt/trn_rl_repo.
"""

import sys

if "/opt/trn_rl_repo" not in sys.path:
    sys.path.insert(0, "/opt/trn_rl_repo")

import numpy as np

N, E, H = 8192, 256, 64          # atoms, embedding, head size
NSEG, SEG = 128, 64              # molecules, atoms per molecule
NCORES = 8
RPC = N // NCORES                # rows per core (1024 = 16 molecules)
NT = RPC // 128                  # 128-row tiles per core (2 molecules each)
HF = NT // 2                     # tiles per half
EC = E // 128                    # embedding chunks of 128
EPS = 5e-5                       # sqrt clamp bias (covers d2 roundoff)

_cache = {}


def _build_nc():
    import concourse.bacc as bacc
    import concourse.tile as tile
    from concourse import mybir

    f32 = mybir.dt.float32
    f16 = mybir.dt.float16
    AF = mybir.ActivationFunctionType

    nc = bacc.Bacc(None, target_bir_lowering=False, debug=False)

    # zat = [invr0*[z2,1,-2z]; 16; 16a], zbt = [invr0*[1,z2,z]; 16; -16a]
    # (a = +-1 per 64-molecule): rows 5-6 add 256 - 256*a_i*a_j to d2',
    # i.e. +512 for cross-molecule pairs -> dexp underflows to 0 = mask.
    zat_d = nc.dram_tensor("zat", [7, NT, 128], f32, kind="ExternalInput")
    zbt_d = nc.dram_tensor("zbt", [7, NT, 128], f32, kind="ExternalInput")
    xt_d = nc.dram_tensor("xt", [128, EC, RPC], f16, kind="ExternalInput")
    # w: [:, c, 0:64] = Wq^T*H^-0.5, [:, c, 64:128] = Wk^T, [:, c, 128:192] = Wv^T
    w_d = nc.dram_tensor("w", [128, EC, 192], f16, kind="ExternalInput")
    y_d = nc.dram_tensor("y", [RPC, H], f16, kind="ExternalOutput")

    with tile.TileContext(nc) as tc:
        with (
            tc.tile_pool(name="consts", bufs=1) as consts,
            tc.tile_pool(name="pd", bufs=1, space="PSUM") as pd,
            tc.tile_pool(name="pst", bufs=2, space="PSUM") as pst,
            tc.tile_pool(name="pmm", bufs=2, space="PSUM") as pmm,
            tc.tile_pool(name="po", bufs=2, space="PSUM") as po,
        ):
            # --- tiny on-device constants (no DMA) ---
            # Half-masked ones columns for block-local rowsums.
            mlo = consts.tile([128, 1], f16, tag="mlo")
            nc.gpsimd.memset(mlo[0:SEG, :], 1.0)
            nc.gpsimd.memset(mlo[SEG:128, :], 0.0)
            mhi = consts.tile([128, 1], f16, tag="mhi")
            nc.gpsimd.memset(mhi[0:SEG, :], 0.0)
            nc.gpsimd.memset(mhi[SEG:128, :], 1.0)
            wu_l = consts.tile([1, 128], f16, tag="wu_l")
            nc.gpsimd.memset(wu_l, 0.0)
            wu_r = consts.tile([1, 512], f16, tag="wu_r")
            nc.gpsimd.memset(wu_r, 0.0)
            sq_in = consts.tile([1, 1], f32, tag="sq_in")
            nc.gpsimd.memset(sq_in, 1.0)
            sq_out = consts.tile([1, 1], f32, tag="sq_out")
            eps_b = consts.tile([128, 1], f32, tag="eps_b")
            nc.gpsimd.memset(eps_b, EPS)

            # --- input DMAs: zat/zbt first (dist chain gates on them) ---
            zat = consts.tile([7, NT, 128], f32, tag="zat")
            nc.sync.dma_start(out=zat, in_=zat_d[:, :, :])
            zbt = consts.tile([7, NT, 128], f32, tag="zbt")
            nc.scalar.dma_start(out=zbt, in_=zbt_d[:, :, :])
            xt = consts.tile([128, EC, RPC], f16, tag="xt")
            nc.sync.dma_start(out=xt, in_=xt_d[:, :, :])
            w_sb = consts.tile([128, EC, 192], f16, tag="w")
            nc.scalar.dma_start(out=w_sb, in_=w_d[:, :, :])

            # Preload the ln/exp ACT table during the DMA wait (Ln lives
            # only in natural_log_exp_and_others, which also has Exp --
            # the whole kernel then needs no further table switches).
            nc.scalar.activation(out=sq_out, in_=sq_in, func=AF.Ln)

            # --- PE warmup: dense f16 matmuls (shared stationary) while
            # waiting on input DMAs, so the HAM clock gate may open. ---
            with tc.high_priority():
                for i in range(5):
                    wu_ps = pmm.tile([128, 512], f32, tag="mi", name=f"wu{i}")
                    nc.tensor.matmul(wu_ps, lhsT=wu_l, rhs=wu_r, start=True, stop=True)

                # --- dist: d2 tiles via K=5 matmuls, then
                # dist = exp(0.5*ln(invr0^2*d2 + eps)), dexp = exp(-dist).
                d_ps = pd.tile([128, NT, 128], f32, tag="d")
                for t in range(NT):
                    nc.tensor.matmul(
                        d_ps[:, t, :],
                        lhsT=zat[:, t, :],
                        rhs=zbt[:, t, :],
                        start=True,
                        stop=True,
                    )
                lnd = consts.tile([128, NT, 128], f32, tag="lnd")
                nc.scalar.activation(out=lnd, in_=d_ps, func=AF.Ln, bias=eps_b)
                dist = consts.tile([128, NT, 128], f32, tag="dist")
                nc.scalar.activation(out=dist, in_=lnd, func=AF.Exp, scale=0.5)
                dexp = consts.tile([128, NT, 128], f16, tag="dexp")
                nc.scalar.activation(out=dexp, in_=dist, func=AF.Exp, scale=-1.0)

            # --- Q/K projections, stacked [Q^T; K^T] in one psum; split
            # into separate qt/kt tiles (matmul operands need base 0). ---
            qt = consts.tile([64, RPC], f16, tag="qt")
            kt = consts.tile([64, RPC], f16, tag="kt")
            for h in range(2):
                hs = slice(h * 512, (h + 1) * 512)
                p = pmm.tile([128, 512], f32, tag="mi", name=f"qk{h}")
                for c in range(EC):
                    nc.tensor.matmul(
                        p,
                        lhsT=w_sb[:, c, 0:128],
                        rhs=xt[:, c, hs],
                        start=(c == 0),
                        stop=(c == EC - 1),
                    )
                nc.vector.tensor_copy(out=kt[:, hs], in_=p[64:128, :])
                nc.vector.tensor_copy(out=qt[:, hs], in_=p[0:64, :])

            # --- V projection, grouped 4 tiles per psum bank ---
            v_sb = consts.tile([128, NT, H], f16, tag="v")
            for g in range(2):
                vp = pmm.tile([128, HF, H], f32, tag="mi", name=f"v{g}")
                for tl in range(HF):
                    t = g * HF + tl
                    for c in range(EC):
                        nc.tensor.matmul(
                            vp[:, tl, :],
                            lhsT=xt[:, c, t * 128 : (t + 1) * 128],
                            rhs=w_sb[:, c, 128:192],
                            start=(c == 0),
                            stop=(c == EC - 1),
                        )
                nc.vector.tensor_copy(out=v_sb[:, g * HF : (g + 1) * HF, :], in_=vp)

            # --- scores (transposed) + mask, exp, decay-mul, rowsums,
            # PV, normalize -- per half of 4 tiles. ---
            eT = consts.tile([128, NT, 128], f16, tag="eT")
            weiT = consts.tile([128, NT, 128], f16, tag="weiT")
            rinv = consts.tile([128, NT], f32, tag="rinv")
            y_sb = consts.tile([128, NT, H], f16, tag="y")
            y_r = y_d.rearrange("(t p) h -> p t h", p=128)

            st_ps = [None] * 2
            for h in range(2):
                st_ps[h] = pst.tile([128, HF, 128], f32, tag="st", name=f"st{h}")
                for tl in range(HF):
                    t = h * HF + tl
                    rt = slice(t * 128, (t + 1) * 128)
                    nc.tensor.matmul(
                        st_ps[h][:, tl, :],
                        lhsT=kt[:, rt],
                        rhs=qt[:, rt],
                        start=True,
                        stop=True,
                    )

            for h in range(2):
                hs = slice(h * HF, (h + 1) * HF)
                nc.scalar.activation(out=eT[:, hs, :], in_=st_ps[h], func=AF.Exp)
                nc.vector.tensor_mul(
                    out=weiT[:, hs, :], in0=eT[:, hs, :], in1=dexp[:, hs, :]
                )
                # Block-local rowsums: r_i = sum_{j in block(i)} e[i,j],
                # two half-masked N=1 matmuls per tile.
                rs = pst.tile([128, HF], f32, tag="st", name=f"rs{h}")
                for tl in range(HF):
                    t = h * HF + tl
                    nc.tensor.matmul(
                        rs[0:SEG, tl : tl + 1],
                        lhsT=eT[:, t, 0:SEG],
                        rhs=mlo,
                        start=True,
                        stop=True,
                    )
                    nc.tensor.matmul(
                        rs[SEG:128, tl : tl + 1],
                        lhsT=eT[:, t, SEG:128],
                        rhs=mhi,
                        start=True,
                        stop=True,
                    )
                nc.vector.reciprocal(out=rinv[:, hs], in_=rs)
                o_ps = po.tile([128, HF, H], f32, tag="o", name=f"o{h}")
                for tl in range(HF):
                    t = h * HF + tl
                    nc.tensor.matmul(
                        o_ps[:, tl, :],
                        lhsT=weiT[:, t, :],
                        rhs=v_sb[:, t, :],
                        start=True,
                        stop=True,
                    )
                rb = rinv[:, hs].unsqueeze(2).broadcast_to([128, HF, H])
                nc.vector.tensor_mul(out=y_sb[:, hs, :], in0=o_ps, in1=rb)
                nc.sync.dma_start(out=y_r[:, hs, :], in_=y_sb[:, hs, :])

    nc.compile()
    return nc


def _get_nc():
    if "nc" not in _cache:
        _cache["nc"] = _build_nc()
    return _cache["nc"]


def _prepare_in_maps(X, Z, Wk, Wq, Wv, invr0):
    X = np.ascontiguousarray(X, dtype=np.float32)
    Z = np.ascontiguousarray(Z, dtype=np.float32)
    # [128, EC, N] fp16: partition p, chunk c -> X^T row c*128+p.
    xt_full = np.ascontiguousarray(
        X.T.reshape(EC, 128, N).transpose(1, 0, 2).astype(np.float16)
    )

    # invr0 folded into both dist operands: psum d2' = invr0^2 * d2, so
    # sqrt(d2' + eps) = invr0*dist and the decay is exp(-1.0 * x).
    inv = np.float32(np.asarray(invr0).reshape(-1)[0])
    z2 = np.sum(Z * Z, axis=-1)
    ones = np.ones(N, dtype=np.float32)
    zt = np.ascontiguousarray(Z.T)
    # a = +-1 per 64-atom molecule: rows 5-6 contribute 256 - 256*a_i*a_j
    # to d2' (0 same-molecule, +512 cross -> decay underflows to 0).
    a = np.where((np.arange(N) % 128) < SEG, 1.0, -1.0).astype(np.float32)
    zat_full = np.concatenate(
        [inv * z2[None], inv * ones[None], inv * -2.0 * zt, 16.0 * ones[None], 16.0 * a[None]],
        axis=0,
    ).astype(np.float32)
    zbt_full = np.concatenate(
        [inv * ones[None], inv * z2[None], inv * zt, 16.0 * ones[None], -16.0 * a[None]],
        axis=0,
    ).astype(np.float32)

    scale = np.float32(H) ** np.float32(-0.5)
    # w: [128, EC, 192] = [Wq^T*scale | Wk^T | Wv^T] per chunk
    wq = (Wq.T * scale).astype(np.float32).reshape(EC, 128, H)
    wk = Wk.T.astype(np.float32).reshape(EC, 128, H)
    wv = Wv.T.astype(np.float32).reshape(EC, 128, H)
    w_full = np.ascontiguousarray(
        np.concatenate([wq, wk, wv], axis=2).astype(np.float16)
    )  # [EC, 128, 192] -> want [128, EC, 192]
    w_full = np.ascontiguousarray(w_full.transpose(1, 0, 2))

    in_maps = []
    for d in range(NCORES):
        s, e = d * RPC, (d + 1) * RPC
        in_maps.append(
            {
                "zat": np.ascontiguousarray(zat_full[:, s:e].reshape(7, NT, 128)),
                "zbt": np.ascontiguousarray(zbt_full[:, s:e].reshape(7, NT, 128)),
                "xt": np.ascontiguousarray(xt_full[:, :, s:e]),
                "w": w_full,
            }
        )
    return in_maps


def _run(in_maps, trace=False, **kwargs):
    from concourse.bass_utils import run_bass_kernel_spmd

    nc = _get_nc()
    return run_bass_kernel_spmd(nc, in_maps, list(range(NCORES)), trace=trace, **kwargs)


def _numpy_fallback(X, Z, Wk, Wq, Wv, invr0, ptr):
    """Reference-exact fallback for ptr layouts other than 128 x 64."""
    X = np.asarray(X, dtype=np.float32)
    Z = np.asarray(Z, dtype=np.float32)
    n = X.shape[0]
    K = X @ Wk.T
    Q = X @ Wq.T
    V = X @ Wv.T
    seg = np.searchsorted(np.asarray(ptr)[1:], np.arange(n), side="right")
    out = np.zeros((n, Wk.shape[0]), dtype=np.float32)
    inv = float(np.asarray(invr0).reshape(-1)[0])
    hs = Wk.shape[0] ** -0.5
    for s in np.unique(seg):
        idx = np.nonzero(seg == s)[0]
        q, k, v, z = Q[idx], K[idx], V[idx], Z[idx]
        wei = (q @ k.T) * hs
        wei = wei - wei.max(axis=-1, keepdims=True)
        wei = np.exp(wei)
        wei /= wei.sum(axis=-1, keepdims=True)
        d2 = np.maximum(
            (z * z).sum(-1)[:, None] + (z * z).sum(-1)[None, :] - 2.0 * (z @ z.T), 0.0
        )
        dist = np.sqrt(np.where(d2 > 0, d2, 1.0)) * (d2 > 0)
        wei = wei * np.exp(-inv * dist)
        out[idx] = wei @ v
    return out


def kernel(X, Z, Wk, Wq, Wv, invr0, ptr):
    ptr = np.asarray(ptr)
    if not (
        X.shape == (N, E)
        and Wk.shape == (H, E)
        and ptr.shape == (NSEG + 1,)
        and np.array_equal(ptr, np.arange(NSEG + 1, dtype=ptr.dtype) * SEG)
    ):
        return _numpy_fallback(X, Z, Wk, Wq, Wv, invr0, ptr)

    in_maps = _prepare_in_maps(X, Z, Wk, Wq, Wv, invr0)
    res = _run(in_maps, trace=False)
    out = np.empty((N, H), dtype=np.float32)
    for d in range(NCORES):
        out[d * RPC : (d + 1) * RPC] = res.results[d]["y"].astype(np.float32)
    return out


# revision 28
# speedup vs baseline: 1.2889x; 1.0202x over previous
"""Trainium2 Bass kernel for nn_DistHead (block-diagonal molecule attention).

out = softmax_blockdiag(Q K^T / sqrt(H)) * exp(-invr0 * cdist(Z, Z)) @ V
with Q/K/V = X @ W{q,k,v}^T, block-diagonal over 128 molecules of 64 atoms.

Sharding: 16 whole molecules (1024 rows) per core across 8 cores --
perfectly parallel, zero cross-core communication.

Key structure (v1 rewrite):
- Scores computed TRANSPOSED (lhsT=K^T, rhs=Q^T) so exp(ST) is directly
  the PV stationary -- no PE transposes, no psum->sbuf weight copies.
- Block-diag mask: one K=1 matmul adding sig_i*sig_j (+-256) to each
  score tile, plus bias=-256 folded into the exp -- off-block scores
  underflow to exactly 0.
- Row sums of exp(S) as N=1 matmuls against a ones column (PE), not DVE
  reduces.
- dist chain: sqrt(invr0^2*d2 + eps) via ACT free affine (clamp fused),
  then exp(-x). Sqrt table preloaded by a dummy activation during the
  DMA wait.
- Q/K projections stacked into one [128,512] psum (stationary [Wq|Wk]),
  halving the psum->sbuf cast traffic.
- PE warmed up by dummy matmuls during the input-DMA wait so real
  matmuls run at 2.4 GHz.
- 3 input DMA issues total (zz, xt, w); sig/ones constants via memset.

Self-contained: hardcodes shapes from the problem spec; only imports
concourse from /opt/trn_rl_repo.
"""

import sys

if "/opt/trn_rl_repo" not in sys.path:
    sys.path.insert(0, "/opt/trn_rl_repo")

import numpy as np

N, E, H = 8192, 256, 64          # atoms, embedding, head size
NSEG, SEG = 128, 64              # molecules, atoms per molecule
NCORES = 8
RPC = N // NCORES                # rows per core (1024 = 16 molecules)
NT = RPC // 128                  # 128-row tiles per core (2 molecules each)
HF = NT // 2                     # tiles per half
EC = E // 128                    # embedding chunks of 128
EPS = 5e-5                       # sqrt clamp bias (covers d2 roundoff)

_cache = {}


def _build_nc():
    import concourse.bacc as bacc
    import concourse.tile as tile
    from concourse import mybir

    f32 = mybir.dt.float32
    f16 = mybir.dt.float16
    AF = mybir.ActivationFunctionType

    nc = bacc.Bacc(None, target_bir_lowering=False, debug=False)

    # Block-diagonal dist operands, 4 tiles per matmul group (K=28):
    # zab[7*tl+c, g*128+m] = zat_c of tile 4g+tl; zbb[7*tl+c, n] =
    # zbt_c of tile 4g+tl on its own 128 cols, 0 elsewhere.
    # Rows per tile: invr0*[z2,1,-2z]+[16,16a] vs invr0*[1,z2,z]+[16,-16a]
    # (a = +-1 per 64-molecule): rows 5-6 add 256 - 256*a_i*a_j to d2',
    # i.e. +512 for cross-molecule pairs -> dexp underflows to 0 = mask.
    zab_d = nc.dram_tensor("zab", [7 * HF, 2, 128], f32, kind="ExternalInput")
    zbb_d = nc.dram_tensor("zbb", [7 * HF, RPC], f32, kind="ExternalInput")
    xt_d = nc.dram_tensor("xt", [128, EC, RPC], f16, kind="ExternalInput")
    # w: [:, c, 0:64] = Wq^T*H^-0.5, [:, c, 64:128] = Wk^T, [:, c, 128:192] = Wv^T
    w_d = nc.dram_tensor("w", [128, EC, 192], f16, kind="ExternalInput")
    y_d = nc.dram_tensor("y", [RPC, H], f16, kind="ExternalOutput")

    with tile.TileContext(nc) as tc:
        with (
            tc.tile_pool(name="consts", bufs=1) as consts,
            tc.tile_pool(name="pd", bufs=1, space="PSUM") as pd,
            tc.tile_pool(name="pst", bufs=2, space="PSUM") as pst,
            tc.tile_pool(name="pmm", bufs=2, space="PSUM") as pmm,
            tc.tile_pool(name="po", bufs=2, space="PSUM") as po,
        ):
            # --- tiny on-device constants (no DMA) ---
            # Half-masked ones columns for block-local rowsums.
            mlo = consts.tile([128, 1], f16, tag="mlo")
            nc.gpsimd.memset(mlo[0:SEG, :], 1.0)
            nc.gpsimd.memset(mlo[SEG:128, :], 0.0)
            mhi = consts.tile([128, 1], f16, tag="mhi")
            nc.gpsimd.memset(mhi[0:SEG, :], 0.0)
            nc.gpsimd.memset(mhi[SEG:128, :], 1.0)
            sq_in = consts.tile([1, 1], f32, tag="sq_in")
            nc.gpsimd.memset(sq_in, 1.0)
            sq_out = consts.tile([1, 1], f32, tag="sq_out")
            eps_b = consts.tile([128, 1], f32, tag="eps_b")
            nc.gpsimd.memset(eps_b, EPS)

            # --- input DMAs: z operands first (dist chain gates on them) ---
            zbb = consts.tile([7 * HF, RPC], f32, tag="zbb")
            nc.sync.dma_start(out=zbb, in_=zbb_d[:, :])
            xt = consts.tile([128, EC, RPC], f16, tag="xt")
            nc.sync.dma_start(out=xt, in_=xt_d[:, :, :])
            zab = consts.tile([7 * HF, 2, 128], f32, tag="zab")
            nc.scalar.dma_start(out=zab, in_=zab_d[:, :, :])
            w_sb = consts.tile([128, EC, 192], f16, tag="w")
            nc.scalar.dma_start(out=w_sb, in_=w_d[:, :, :])

            # Preload the sqrt ACT table during the DMA wait (after the
            # Scalar-queue DMA issues so it doesn't delay them).
            nc.scalar.activation(out=sq_out, in_=sq_in, func=AF.Sqrt)

            with tc.high_priority():
                # --- dist: d2 via 2 block-diagonal K=28 matmuls, then
                # dist = sqrt(invr0^2*d2 + eps) and dexp = exp(-dist).
                d_ps = pd.tile([128, NT, 128], f32, tag="d")
                for g in range(2):
                    nc.tensor.matmul(
                        d_ps[:, g * HF : (g + 1) * HF, :],
                        lhsT=zab[:, g, :],
                        rhs=zbb[:, g * 512 : (g + 1) * 512],
                        start=True,
                        stop=True,
                    )
                dist = consts.tile([128, NT, 128], f32, tag="dist")
                nc.scalar.activation(out=dist, in_=d_ps, func=AF.Sqrt, bias=eps_b)
                dexp = consts.tile([128, NT, 128], f16, tag="dexp")
                nc.scalar.activation(out=dexp, in_=dist, func=AF.Exp, scale=-1.0)

            # --- Q/K projections, stacked [Q^T; K^T] in one psum; split
            # into separate qt/kt tiles (matmul operands need base 0). ---
            qt = consts.tile([64, RPC], f16, tag="qt")
            kt = consts.tile([64, RPC], f16, tag="kt")
            for h in range(2):
                hs = slice(h * 512, (h + 1) * 512)
                p = pmm.tile([128, 512], f32, tag="mi", name=f"qk{h}")
                for c in range(EC):
                    nc.tensor.matmul(
                        p,
                        lhsT=w_sb[:, c, 0:128],
                        rhs=xt[:, c, hs],
                        start=(c == 0),
                        stop=(c == EC - 1),
                    )
                nc.vector.tensor_copy(out=kt[:, hs], in_=p[64:128, :])
                nc.vector.tensor_copy(out=qt[:, hs], in_=p[0:64, :])

            # --- V projections (PE) interleaved with transposed score
            # matmuls so the PE fills its cast-wait gaps. ---
            v_sb = consts.tile([128, NT, H], f16, tag="v")
            vp = [None] * 2
            st_ps = [None] * 2
            for h in range(2):
                vp[h] = pmm.tile([128, HF, H], f32, tag="mi", name=f"v{h}")
                for tl in range(HF):
                    t = h * HF + tl
                    for c in range(EC):
                        nc.tensor.matmul(
                            vp[h][:, tl, :],
                            lhsT=xt[:, c, t * 128 : (t + 1) * 128],
                            rhs=w_sb[:, c, 128:192],
                            start=(c == 0),
                            stop=(c == EC - 1),
                        )
                st_ps[h] = pst.tile([128, HF, 128], f32, tag="st", name=f"st{h}")
                for tl in range(HF):
                    t = h * HF + tl
                    rt = slice(t * 128, (t + 1) * 128)
                    nc.tensor.matmul(
                        st_ps[h][:, tl, :],
                        lhsT=kt[:, rt],
                        rhs=qt[:, rt],
                        start=True,
                        stop=True,
                    )

            eT = consts.tile([128, NT, 128], f16, tag="eT")
            weiT = consts.tile([128, NT, 128], f16, tag="weiT")
            rinv = consts.tile([128, NT], f32, tag="rinv")
            y_sb = consts.tile([128, NT, H], f16, tag="y")
            y_r = y_d.rearrange("(t p) h -> p t h", p=128)

            for h in range(2):
                hs = slice(h * HF, (h + 1) * HF)
                nc.scalar.activation(out=eT[:, hs, :], in_=st_ps[h], func=AF.Exp)
                # V psum -> sbuf on ACT (Copy is in every table set);
                # keeps the DVE free for the qt/kt casts.
                nc.scalar.copy(out=v_sb[:, hs, :], in_=vp[h])
                nc.vector.tensor_mul(
                    out=weiT[:, hs, :], in0=eT[:, hs, :], in1=dexp[:, hs, :]
                )
                # Block-local rowsums: r_i = sum_{j in block(i)} e[i,j],
                # two half-masked N=1 matmuls per tile.
                rs = pst.tile([128, HF], f32, tag="st", name=f"rs{h}")
                for tl in range(HF):
                    t = h * HF + tl
                    nc.tensor.matmul(
                        rs[0:SEG, tl : tl + 1],
                        lhsT=eT[:, t, 0:SEG],
                        rhs=mlo,
                        start=True,
                        stop=True,
                    )
                    nc.tensor.matmul(
                        rs[SEG:128, tl : tl + 1],
                        lhsT=eT[:, t, SEG:128],
                        rhs=mhi,
                        start=True,
                        stop=True,
                    )
                nc.vector.reciprocal(out=rinv[:, hs], in_=rs)
                o_ps = po.tile([128, HF, H], f32, tag="o", name=f"o{h}")
                for tl in range(HF):
                    t = h * HF + tl
                    nc.tensor.matmul(
                        o_ps[:, tl, :],
                        lhsT=weiT[:, t, :],
                        rhs=v_sb[:, t, :],
                        start=True,
                        stop=True,
                    )
                rb = rinv[:, hs].unsqueeze(2).broadcast_to([128, HF, H])
                nc.vector.tensor_mul(out=y_sb[:, hs, :], in0=o_ps, in1=rb)
                nc.sync.dma_start(out=y_r[:, hs, :], in_=y_sb[:, hs, :])

    nc.compile()
    return nc


def _get_nc():
    if "nc" not in _cache:
        _cache["nc"] = _build_nc()
    return _cache["nc"]


def _prepare_in_maps(X, Z, Wk, Wq, Wv, invr0):
    X = np.ascontiguousarray(X, dtype=np.float32)
    Z = np.ascontiguousarray(Z, dtype=np.float32)
    # [128, EC, N] fp16: partition p, chunk c -> X^T row c*128+p.
    xt_full = np.ascontiguousarray(
        X.T.reshape(EC, 128, N).transpose(1, 0, 2).astype(np.float16)
    )

    # invr0 folded into both dist operands: psum d2' = invr0^2 * d2, so
    # sqrt(d2' + eps) = invr0*dist and the decay is exp(-1.0 * x).
    inv = np.float32(np.asarray(invr0).reshape(-1)[0])
    z2 = np.sum(Z * Z, axis=-1)
    ones = np.ones(N, dtype=np.float32)
    zt = np.ascontiguousarray(Z.T)
    # a = +-1 per 64-atom molecule: rows 5-6 contribute 256 - 256*a_i*a_j
    # to d2' (0 same-molecule, +512 cross -> decay underflows to 0).
    a = np.where((np.arange(N) % 128) < SEG, 1.0, -1.0).astype(np.float32)
    zat_full = np.concatenate(
        [inv * z2[None], inv * ones[None], inv * -2.0 * zt, 16.0 * ones[None], 16.0 * a[None]],
        axis=0,
    ).astype(np.float32)
    zbt_full = np.concatenate(
        [inv * ones[None], inv * z2[None], inv * zt, 16.0 * ones[None], -16.0 * a[None]],
        axis=0,
    ).astype(np.float32)

    scale = np.float32(H) ** np.float32(-0.5)
    # w: [128, EC, 192] = [Wq^T*scale | Wk^T | Wv^T] per chunk
    wq = (Wq.T * scale).astype(np.float32).reshape(EC, 128, H)
    wk = Wk.T.astype(np.float32).reshape(EC, 128, H)
    wv = Wv.T.astype(np.float32).reshape(EC, 128, H)
    w_full = np.ascontiguousarray(
        np.concatenate([wq, wk, wv], axis=2).astype(np.float16)
    )  # [EC, 128, 192] -> want [128, EC, 192]
    w_full = np.ascontiguousarray(w_full.transpose(1, 0, 2))

    in_maps = []
    for d in range(NCORES):
        s, e = d * RPC, (d + 1) * RPC
        # Block-diagonal dist operands: group g covers tiles 4g..4g+3.
        # zab[7*tl+c, g, :] = zat row c of tile 4g+tl;
        # zbb[7*tl+c, g*512+tl*128 : +128] = zbt row c of tile 4g+tl.
        za = zat_full[:, s:e].reshape(7, NT, 128)
        zb = zbt_full[:, s:e].reshape(7, NT, 128)
        zab = np.empty((7 * HF, 2, 128), dtype=np.float32)
        zbb = np.zeros((7 * HF, RPC), dtype=np.float32)
        for g in range(2):
            for tl in range(HF):
                t = g * HF + tl
                zab[7 * tl : 7 * tl + 7, g, :] = za[:, t, :]
                c0 = g * 512 + tl * 128
                zbb[7 * tl : 7 * tl + 7, c0 : c0 + 128] = zb[:, t, :]
        in_maps.append(
            {
                "zab": np.ascontiguousarray(zab),
                "zbb": np.ascontiguousarray(zbb),
                "xt": np.ascontiguousarray(xt_full[:, :, s:e]),
                "w": w_full,
            }
        )
    return in_maps


def _run(in_maps, trace=False, **kwargs):
    from concourse.bass_utils import run_bass_kernel_spmd

    nc = _get_nc()
    return run_bass_kernel_spmd(nc, in_maps, list(range(NCORES)), trace=trace, **kwargs)


def _numpy_fallback(X, Z, Wk, Wq, Wv, invr0, ptr):
    """Reference-exact fallback for ptr layouts other than 128 x 64."""
    X = np.asarray(X, dtype=np.float32)
    Z = np.asarray(Z, dtype=np.float32)
    n = X.shape[0]
    K = X @ Wk.T
    Q = X @ Wq.T
    V = X @ Wv.T
    seg = np.searchsorted(np.asarray(ptr)[1:], np.arange(n), side="right")
    out = np.zeros((n, Wk.shape[0]), dtype=np.float32)
    inv = float(np.asarray(invr0).reshape(-1)[0])
    hs = Wk.shape[0] ** -0.5
    for s in np.unique(seg):
        idx = np.nonzero(seg == s)[0]
        q, k, v, z = Q[idx], K[idx], V[idx], Z[idx]
        wei = (q @ k.T) * hs
        wei = wei - wei.max(axis=-1, keepdims=True)
        wei = np.exp(wei)
        wei /= wei.sum(axis=-1, keepdims=True)
        d2 = np.maximum(
            (z * z).sum(-1)[:, None] + (z * z).sum(-1)[None, :] - 2.0 * (z @ z.T), 0.0
        )
        dist = np.sqrt(np.where(d2 > 0, d2, 1.0)) * (d2 > 0)
        wei = wei * np.exp(-inv * dist)
        out[idx] = wei @ v
    return out


def kernel(X, Z, Wk, Wq, Wv, invr0, ptr):
    ptr = np.asarray(ptr)
    if not (
        X.shape == (N, E)
        and Wk.shape == (H, E)
        and ptr.shape == (NSEG + 1,)
        and np.array_equal(ptr, np.arange(NSEG + 1, dtype=ptr.dtype) * SEG)
    ):
        return _numpy_fallback(X, Z, Wk, Wq, Wv, invr0, ptr)

    in_maps = _prepare_in_maps(X, Z, Wk, Wq, Wv, invr0)
    res = _run(in_maps, trace=False)
    out = np.empty((N, H), dtype=np.float32)
    for d in range(NCORES):
        out[d * RPC : (d + 1) * RPC] = res.results[d]["y"].astype(np.float32)
    return out
